# revision 1
# baseline (speedup 1.0000x reference)
"""LIF spike-train scan (nn_LIFSpike) on 8 TRN2 NeuronCores.

Reference semantics (fp32, bit-exact):
    u_t = TAU * u_{t-1} * (1 - o_{t-1}) + x_t ;  o_t = (u_t > VTH)
with u_{-1} = o_{-1} = 0, scanned over the trailing time dim (T=50).

Sharding: pure data parallel — the 16*64*32*32 = 1,048,576 spatial elements
are split evenly across 8 cores (131,072 each); the time scan runs on-chip.

On-chip layout per core: tiles of [128 partitions, F spatial, 50 time], time
scanned sequentially with all-spatial-parallel vector ops.  Per step:
    g   = u * [u <= VTH]          (scalar_tensor_tensor / fused)
    u'  = TAU * g + x_t           (scalar_tensor_tensor / fused)
    o_t = [u' > VTH]              (tensor_scalar is_gt)
which reproduces the reference rounding exactly: round(TAU*u) then *{0,1}
then round(+x) == round(TAU*(u*{0,1})) + x for each branch.
"""

import os
import numpy as np

import concourse.bass as bass
import concourse.bacc as bacc
import concourse.tile as tile
from concourse import mybir
from concourse.bass_utils import run_bass_kernel_spmd

TAU = 0.3
VTH = 0.3

T = 50
S_FULL = 16 * 64 * 32 * 32          # 1,048,576 spatial elements
N_CORES = 8
S_CORE = S_FULL // N_CORES          # 131,072
P = 128                             # SBUF partitions
F = 128                             # spatial elements per partition per tile
NB = S_CORE // (P * F)              # tiles per core

USE_FUSED = os.environ.get("LIF_FUSED", "1") == "1"
DMA_ENGINE = os.environ.get("LIF_DMA", "sync")      # sync | gpsimd
SPIKE_ENGINE = os.environ.get("LIF_SPIKE", "gpsimd")  # vector | gpsimd
SPLIT_DMA = int(os.environ.get("LIF_SPLIT_DMA", "1"))  # x/o DMA split factor

# results of the last run (for test.py to inspect trace/exec time)
LAST_RESULTS = None

_FUSED_OP = None


def _get_fused_op():
    """Register the fused gated-leak op: out = select(VTH >= u, u, 0)*TAU + x.

    One DVE instruction per scan step instead of two scalar_tensor_tensor
    passes.  Registered at runtime into concourse.dve_ops' module-level
    registry (OPS / CUSTOM_DVE_SPECS / opcode map), which is all the
    table-gen path reads."""
    global _FUSED_OP
    if _FUSED_OP is not None:
        return _FUSED_OP
    import concourse.dve_ops as dve_ops
    from concourse.dve_spec import Spec, Src0, Src1, C0, C1, Zero, select, lower
    from concourse.dve_uop import DveOpSpec

    name = "LIF_GATED_LEAK_ANT"
    spec = Spec(
        body=select(C0 >= Src0, Src0, Zero) * C1 + Src1,
        reference=lambda in0, in1, s0, s1, imm2: (
            np.where(s0 >= in0, in0, np.float32(0.0)).astype(np.float32) * np.float32(s1)
        ).astype(np.float32)
        + in1,
    )
    existing = {op.name for op in dve_ops.OPS}
    if name not in existing:
        row = dve_ops._CUSTOM_DVE_ROW_BASE + len(dve_ops.OPS)
        assert row < 0x20, "custom-DVE opcode row overflow"
        # pin the sha to what lower() actually produces (self-consistent)
        shas = {}
        for ver in ("v3", "v4"):
            uops = lower(spec, ver=ver)
            shas[ver] = DveOpSpec(name=name, opcode=row, uops=uops, rd1_en=True).sha(ver)
        op = dve_ops.DveOp(name, spec, subdim=False, uops_sha=shas)
        dve_ops.OPS.append(op)
        dve_ops.CUSTOM_DVE_SPECS[name] = spec
        dve_ops._SUB_OPCODE_FOR_NAME[name] = row
        _FUSED_OP = op
    else:
        _FUSED_OP = next(op for op in dve_ops.OPS if op.name == name)
    return _FUSED_OP


def _build_program():
    f32 = mybir.dt.float32
    nc = bacc.Bacc("TRN2", target_bir_lowering=False, debug=False)

    x_d = nc.dram_tensor("x", [NB, P, F, T], f32, kind="ExternalInput").ap()
    o_d = nc.dram_tensor("o", [NB, P, F, T], f32, kind="ExternalOutput").ap()

    fused = _get_fused_op() if USE_FUSED else None

    with tile.TileContext(nc) as tc:
        with (
            tc.tile_pool(name="xp", bufs=3) as xp,
            tc.tile_pool(name="op", bufs=2) as op_,
            tc.tile_pool(name="up", bufs=2) as up,
            tc.tile_pool(name="gp", bufs=2) as gp,
        ):
            dma = nc.sync if DMA_ENGINE == "sync" else nc.gpsimd
            spike_eng = nc.gpsimd if SPIKE_ENGINE == "gpsimd" else nc.vector
            fc = F // SPLIT_DMA  # spatial chunk per DMA
            for b in range(NB):
                xt = xp.tile([P, F, T], f32)
                for s in range(SPLIT_DMA):
                    dma.dma_start(
                        out=xt[:, s * fc:(s + 1) * fc, :],
                        in_=x_d[b][:, s * fc:(s + 1) * fc, :],
                    )
                ot = op_.tile([P, F, T], f32)

                u = None
                for t in range(T):
                    u_new = up.tile([P, F], f32)
                    if t == 0:
                        # u_0 = x_0 (carry is zero)
                        nc.vector.tensor_copy(u_new[:], xt[:, :, t])
                    elif fused is not None:
                        nc.vector._custom_dve(
                            fused,
                            out=u_new[:],
                            in0=u[:],
                            in1=xt[:, :, t],
                            s0=VTH,
                            s1=TAU,
                        )
                    else:
                        g = gp.tile([P, F], f32)
                        nc.vector.scalar_tensor_tensor(
                            g[:], u[:], VTH, u[:],
                            mybir.AluOpType.is_le, mybir.AluOpType.mult,
                        )
                        nc.vector.scalar_tensor_tensor(
                            u_new[:], g[:], TAU, xt[:, :, t],
                            mybir.AluOpType.mult, mybir.AluOpType.add,
                        )
                    u = u_new
                    spike_eng.tensor_scalar(
                        ot[:, :, t], u[:], VTH, None, mybir.AluOpType.is_gt
                    )

                for s in range(SPLIT_DMA):
                    dma.dma_start(
                        out=o_d[b][:, s * fc:(s + 1) * fc, :],
                        in_=ot[:, s * fc:(s + 1) * fc, :],
                    )
    nc.compile()
    return nc


def _make_runner(nc):
    """Jitted 8-core runner over device-resident buffers (for benchmarking).

    Mirrors bass2jax.run_bass_via_pjrt's shard_map construction but without
    donation, so input buffers stay alive across repeated timed calls.  The
    kernel writes every output element, so the output-seed buffer contents
    are irrelevant."""
    import jax
    import jax.numpy as jnp
    from jax.sharding import Mesh, PartitionSpec, NamedSharding
    from jax.experimental.shard_map import shard_map
    from concourse import bass2jax, mybir as _mybir

    bass2jax.install_neuronx_cc_hook()

    in_names, out_names, out_avals = [], [], []
    for alloc in nc.m.functions[0].allocations:
        if not isinstance(alloc, mybir.MemoryLocationSet):
            continue
        name = alloc.memorylocations[0].name
        if alloc.kind == "ExternalInput":
            if nc.partition_id_tensor is None or name != nc.partition_id_tensor.name:
                in_names.append(name)
        elif alloc.kind == "ExternalOutput":
            out_names.append(name)
            out_avals.append(
                jax.core.ShapedArray(tuple(alloc.tensor_shape), _mybir.dt.np(alloc.dtype))
            )
    all_in = list(in_names) + list(out_names)
    if nc.partition_id_tensor is not None:
        all_in.append(nc.partition_id_tensor.name)

    def _body(*args):
        operands = list(args)
        if nc.partition_id_tensor is not None:
            operands.append(bass2jax.partition_id_tensor())
        return tuple(
            bass2jax._bass_exec_p.bind(
                *operands,
                out_avals=tuple(out_avals),
                in_names=tuple(all_in),
                out_names=tuple(out_names),
                lowering_input_output_aliases=(),
                sim_require_finite=True,
                sim_require_nnan=True,
                nc=nc,
            )
        )

    devices = jax.devices()[:N_CORES]
    mesh = Mesh(np.asarray(devices), ("core",))
    n_ops = len(in_names) + len(out_names)
    fn = jax.jit(
        shard_map(
            _body,
            mesh=mesh,
            in_specs=(PartitionSpec("core"),) * n_ops,
            out_specs=(PartitionSpec("core"),) * len(out_names),
            check_rep=False,
        ),
        keep_unused=True,
    )
    sh = NamedSharding(mesh, PartitionSpec("core"))
    return fn, sh, out_avals


def bench(x, iters=10):
    """Compile once, device_put inputs, time repeated executions."""
    import time as _time
    import jax

    x = np.ascontiguousarray(np.asarray(x, dtype=np.float32)).reshape(S_FULL, T)
    nc = _build_program()
    fn, sh, out_avals = _make_runner(nc)
    xg = x.reshape(N_CORES * NB, P, F, T)
    xdev = jax.device_put(xg, sh)
    zdev = jax.device_put(
        np.zeros((N_CORES * out_avals[0].shape[0], *out_avals[0].shape[1:]), np.float32), sh
    )
    # warmup + compile
    out = fn(xdev, zdev)
    jax.block_until_ready(out)
    times = []
    for _ in range(iters):
        t0 = _time.perf_counter()
        out = fn(xdev, zdev)
        jax.block_until_ready(out)
        times.append(_time.perf_counter() - t0)
    arr = np.asarray(out[0]).reshape(S_FULL, T)
    return times, arr


def kernel(x, ksi=None, trace=False):
    """Full-input entry: x [16,64,32,32,50] f32 -> spikes, same shape.
    (ksi is unused by the reference computation.)"""
    global LAST_RESULTS
    x = np.ascontiguousarray(np.asarray(x, dtype=np.float32))
    orig_shape = x.shape
    xf = x.reshape(S_FULL, T)

    nc = _build_program()

    in_maps = [
        {"x": xf[i * S_CORE:(i + 1) * S_CORE].reshape(NB, P, F, T)}
        for i in range(N_CORES)
    ]
    res = run_bass_kernel_spmd(nc, in_maps, list(range(N_CORES)), trace=trace)
    LAST_RESULTS = res

    out = np.empty((S_FULL, T), dtype=np.float32)
    for i in range(N_CORES):
        out[i * S_CORE:(i + 1) * S_CORE] = res.results[i]["o"].reshape(S_CORE, T)
    return out.reshape(orig_shape)



# revision 5
# speedup vs baseline: 1.5862x; 1.5862x over previous
"""LIF spike-train scan (nn_LIFSpike) on 8 TRN2 NeuronCores.

Reference semantics (fp32, bit-exact):
    u_t = TAU * u_{t-1} * (1 - o_{t-1}) + x_t ;  o_t = (u_t > VTH)
with u_{-1} = o_{-1} = 0, scanned over the trailing time dim (T=50).

Sharding: pure data parallel - the 16*64*32*32 = 1,048,576 spatial elements
split evenly across 8 cores (131,072 = 128 partitions x 1024 each).

On-chip layout per core: the time axis is chunked (NC chunks of TC steps);
each chunk tile is [128 partitions, TC, 1024] so every compute instruction
covers the full 1024-element free dim (amortizes the cayman per-instruction
read-write bubble).  The membrane history for a chunk lives in SBUF, so the
spike threshold runs as ONE is_gt instruction per chunk over [128, TC*1024].
Spikes are written as uint8 {0,1} (exact) to quarter the output HBM traffic;
the host converts back to f32.

Per step the membrane update is one fused DVE op:
    u_t = select(VTH >= u_{t-1}, u_{t-1}, 0) * TAU + x_t
which reproduces the reference rounding exactly: round(TAU*u) then *{0,1}
then round(+x) == round(TAU*(u*{0,1})) + x for each branch.  The spike
compare is a strict is_gt (no activation-table approximations anywhere).

All compute is on the Vector (DVE) engine; nothing runs on gpsimd (Q7
software loops are ~15ns/element - two orders of magnitude off DVE).
"""

import os
import numpy as np

import concourse.bass as bass
import concourse.bacc as bacc
import concourse.tile as tile
from concourse import mybir
from concourse.bass_utils import run_bass_kernel_spmd

TAU = 0.3
VTH = 0.3

T = 50
S_FULL = 16 * 64 * 32 * 32          # 1,048,576 spatial elements
N_CORES = 8
S_CORE = S_FULL // N_CORES          # 131,072
P = 128                             # SBUF partitions
F = S_CORE // P                     # 1024 spatial elements per partition

TC = int(os.environ.get("LIF_TC", "10"))            # time-steps per chunk
NC = T // TC                                        # chunks (must divide T)
SPIKE_CHUNK = os.environ.get("LIF_SPIKE_CHUNK", "1") == "1"
O_DT = os.environ.get("LIF_O_DT", "u8")             # u8 | bf16 | f32
X_BUFS = int(os.environ.get("LIF_X_BUFS", "4"))
U_BUFS = int(os.environ.get("LIF_U_BUFS", "3"))
O_BUFS = int(os.environ.get("LIF_O_BUFS", "3"))
# DMA issue queues: sync | scalar | alt (alternate per chunk across both
# HW-DGE rings so neither sequencer saturates)
DMA_Q = os.environ.get("LIF_DMA_Q", "alt")

# results of the last run (for test.py to inspect trace/exec time)
LAST_RESULTS = None

_FUSED_OP = None


def _get_fused_op():
    """Register the fused gated-leak op: out = select(VTH >= u, u, 0)*TAU + x.

    One DVE instruction per scan step instead of two scalar_tensor_tensor
    passes.  Registered at runtime into concourse.dve_ops' module-level
    registry (OPS / CUSTOM_DVE_SPECS / opcode map), which is all the
    table-gen path reads."""
    global _FUSED_OP
    if _FUSED_OP is not None:
        return _FUSED_OP
    import concourse.dve_ops as dve_ops
    from concourse.dve_spec import Spec, Src0, Src1, C0, C1, Zero, select, lower
    from concourse.dve_uop import DveOpSpec

    name = "LIF_GATED_LEAK_ANT"
    spec = Spec(
        body=select(C0 >= Src0, Src0, Zero) * C1 + Src1,
        reference=lambda in0, in1, s0, s1, imm2: (
            np.where(s0 >= in0, in0, np.float32(0.0)).astype(np.float32) * np.float32(s1)
        ).astype(np.float32)
        + in1,
    )
    existing = {op.name for op in dve_ops.OPS}
    if name not in existing:
        row = dve_ops._CUSTOM_DVE_ROW_BASE + len(dve_ops.OPS)
        assert row < 0x20, "custom-DVE opcode row overflow"
        # pin the sha to what lower() actually produces (self-consistent)
        shas = {}
        for ver in ("v3", "v4"):
            uops = lower(spec, ver=ver)
            shas[ver] = DveOpSpec(name=name, opcode=row, uops=uops, rd1_en=True).sha(ver)
        op = dve_ops.DveOp(name, spec, subdim=False, uops_sha=shas)
        dve_ops.OPS.append(op)
        dve_ops.CUSTOM_DVE_SPECS[name] = spec
        dve_ops._SUB_OPCODE_FOR_NAME[name] = row
        _FUSED_OP = op
    else:
        _FUSED_OP = next(op for op in dve_ops.OPS if op.name == name)
    return _FUSED_OP


def _o_mybir_dt():
    return {
        "u8": mybir.dt.uint8,
        "bf16": mybir.dt.bfloat16,
        "f32": mybir.dt.float32,
    }[O_DT]


def _build_program():
    f32 = mybir.dt.float32
    odt = _o_mybir_dt()
    nc = bacc.Bacc("TRN2", target_bir_lowering=False, debug=False)

    x_d = nc.dram_tensor("x", [NC, P, TC, F], f32, kind="ExternalInput").ap()
    o_d = nc.dram_tensor("o", [NC, P, TC, F], odt, kind="ExternalOutput").ap()

    fused = _get_fused_op()

    with tile.TileContext(nc) as tc:
        with (
            tc.tile_pool(name="xp", bufs=X_BUFS) as xp,
            tc.tile_pool(name="up", bufs=U_BUFS) as up,
            tc.tile_pool(name="op", bufs=O_BUFS) as op_,
        ):
            def dma_eng(idx):
                if DMA_Q == "sync":
                    return nc.sync
                if DMA_Q == "scalar":
                    return nc.scalar
                return nc.sync if idx % 2 == 0 else nc.scalar

            u_prev = None  # [P, F] slice of the previous chunk's history
            for c in range(NC):
                xt = xp.tile([P, TC, F], f32)
                dma_eng(c).dma_start(out=xt[:], in_=x_d[c])
                uh = up.tile([P, TC, F], f32)   # membrane history for chunk
                ot = op_.tile([P, TC, F], odt)

                for tl in range(TC):
                    u_new = uh[:, tl, :]
                    if c == 0 and tl == 0:
                        # u_0 = x_0 (zero carry)
                        nc.vector.tensor_copy(u_new, xt[:, 0, :])
                    else:
                        nc.vector._custom_dve(
                            fused,
                            out=u_new,
                            in0=u_prev,
                            in1=xt[:, tl, :],
                            s0=VTH,
                            s1=TAU,
                        )
                    u_prev = u_new
                    if not SPIKE_CHUNK:
                        nc.vector.tensor_scalar(
                            ot[:, tl, :], u_new, VTH, None, mybir.AluOpType.is_gt
                        )
                if SPIKE_CHUNK:
                    # one strict-compare over the whole chunk history
                    nc.vector.tensor_scalar(
                        ot[:], uh[:], VTH, None, mybir.AluOpType.is_gt
                    )
                dma_eng(c + 1).dma_start(out=o_d[c], in_=ot[:])
    nc.compile()
    return nc


def kernel(x, ksi=None, trace=False):
    """Full-input entry: x [16,64,32,32,50] f32 -> spikes, same shape.
    (ksi is unused by the reference computation.)"""
    global LAST_RESULTS
    x = np.ascontiguousarray(np.asarray(x, dtype=np.float32))
    orig_shape = x.shape
    xf = x.reshape(S_FULL, T)

    nc = _build_program()

    # device layout per core: [chunk, partition, t-in-chunk, free-spatial]
    in_maps = []
    for i in range(N_CORES):
        xc = xf[i * S_CORE:(i + 1) * S_CORE]            # [S_CORE, T]
        xd = xc.reshape(P, F, NC, TC).transpose(2, 0, 3, 1)  # [NC, P, TC, F]
        in_maps.append({"x": np.ascontiguousarray(xd)})

    res = run_bass_kernel_spmd(nc, in_maps, list(range(N_CORES)), trace=trace)
    LAST_RESULTS = res

    out = np.empty((S_FULL, T), dtype=np.float32)
    for i in range(N_CORES):
        r = res.results[i]["o"]                          # [NC, P, TC, F]
        oc = np.asarray(r).transpose(1, 3, 0, 2).reshape(S_CORE, T)
        if oc.dtype != np.float32:
            oc = (oc != 0).astype(np.float32) if O_DT == "u8" else oc.astype(np.float32)
        out[i * S_CORE:(i + 1) * S_CORE] = oc
    return out.reshape(orig_shape)


# revision 13
# speedup vs baseline: 1.8342x; 1.1564x over previous
"""LIF spike-train scan (nn_LIFSpike) on 8 TRN2 NeuronCores.

Reference semantics (fp32, bit-exact):
    u_t = TAU * u_{t-1} * (1 - o_{t-1}) + x_t ;  o_t = (u_t > VTH)
with u_{-1} = o_{-1} = 0, scanned over the trailing time dim (T=50).

Sharding: pure data parallel - the 16*64*32*32 = 1,048,576 spatial elements
split evenly across 8 cores (131,072 = 128 partitions x 1024 each).

On-chip layout per core: the time axis is chunked (NC chunks of TC steps);
each chunk tile is [128 partitions, TC, 1024] so every compute instruction
covers the full 1024-element free dim (amortizes the cayman per-instruction
read-write bubble).  The membrane history for a chunk lives in SBUF, so the
spike threshold runs as ONE is_gt instruction per chunk over [128, TC*1024].
Spikes are written as uint8 {0,1} (exact) to quarter the output HBM traffic;
the host converts back to f32.

Per step the membrane update is one fused DVE op:
    u_t = select(VTH >= u_{t-1}, u_{t-1}, 0) * TAU + x_t
which reproduces the reference rounding exactly: round(TAU*u) then *{0,1}
then round(+x) == round(TAU*(u*{0,1})) + x for each branch.  The spike
compare is a strict is_gt (no activation-table approximations anywhere).

All compute is on the Vector (DVE) engine; nothing runs on gpsimd (Q7
software loops are ~15ns/element - two orders of magnitude off DVE).
"""

import os
import numpy as np

import concourse.bass as bass
import concourse.bacc as bacc
import concourse.tile as tile
from concourse import mybir
from concourse.bass_utils import run_bass_kernel_spmd

TAU = 0.3
VTH = 0.3

T = 50
S_FULL = 16 * 64 * 32 * 32          # 1,048,576 spatial elements
N_CORES = 8
S_CORE = S_FULL // N_CORES          # 131,072
P = 128                             # SBUF partitions
F = S_CORE // P                     # 1024 spatial elements per partition

TC = int(os.environ.get("LIF_TC", "2"))             # time-steps per chunk
NC = T // TC                                        # chunks (must divide T)
SPIKE_CHUNK = os.environ.get("LIF_SPIKE_CHUNK", "1") == "1"
O_DT = os.environ.get("LIF_O_DT", "u8")             # u8 | bf16 | f32
X_BUFS = int(os.environ.get("LIF_X_BUFS", "4"))
U_BUFS = int(os.environ.get("LIF_U_BUFS", "3"))
O_BUFS = int(os.environ.get("LIF_O_BUFS", "3"))
# DMA issue queues: sync | scalar | alt (alternate per chunk across both
# HW-DGE rings so neither sequencer saturates)
DMA_Q = os.environ.get("LIF_DMA_Q", "dir")
X_SPLIT = int(os.environ.get("LIF_X_SPLIT", "1"))   # x-DMA slices per chunk
O_SPLIT = int(os.environ.get("LIF_O_SPLIT", "1"))   # o-DMA slices per chunk

# results of the last run (for test.py to inspect trace/exec time)
LAST_RESULTS = None

_FUSED_OP = None


def _get_fused_op():
    """Register the fused gated-leak op: out = select(VTH >= u, u, 0)*TAU + x.

    One DVE instruction per scan step instead of two scalar_tensor_tensor
    passes.  Registered at runtime into concourse.dve_ops' module-level
    registry (OPS / CUSTOM_DVE_SPECS / opcode map), which is all the
    table-gen path reads."""
    global _FUSED_OP
    if _FUSED_OP is not None:
        return _FUSED_OP
    import concourse.dve_ops as dve_ops
    from concourse.dve_spec import Spec, Src0, Src1, C0, C1, Zero, select, lower
    from concourse.dve_uop import DveOpSpec

    name = "LIF_GATED_LEAK_ANT"
    spec = Spec(
        body=select(C0 >= Src0, Src0, Zero) * C1 + Src1,
        reference=lambda in0, in1, s0, s1, imm2: (
            np.where(s0 >= in0, in0, np.float32(0.0)).astype(np.float32) * np.float32(s1)
        ).astype(np.float32)
        + in1,
    )
    existing = {op.name for op in dve_ops.OPS}
    if name not in existing:
        row = dve_ops._CUSTOM_DVE_ROW_BASE + len(dve_ops.OPS)
        assert row < 0x20, "custom-DVE opcode row overflow"
        # pin the sha to what lower() actually produces (self-consistent)
        shas = {}
        for ver in ("v3", "v4"):
            uops = lower(spec, ver=ver)
            shas[ver] = DveOpSpec(name=name, opcode=row, uops=uops, rd1_en=True).sha(ver)
        op = dve_ops.DveOp(name, spec, subdim=False, uops_sha=shas)
        dve_ops.OPS.append(op)
        dve_ops.CUSTOM_DVE_SPECS[name] = spec
        dve_ops._SUB_OPCODE_FOR_NAME[name] = row
        _FUSED_OP = op
    else:
        _FUSED_OP = next(op for op in dve_ops.OPS if op.name == name)
    return _FUSED_OP


def _o_mybir_dt():
    return {
        "u8": mybir.dt.uint8,
        "bf16": mybir.dt.bfloat16,
        "f32": mybir.dt.float32,
    }[O_DT]


def _build_program():
    f32 = mybir.dt.float32
    odt = _o_mybir_dt()
    nc = bacc.Bacc("TRN2", target_bir_lowering=False, debug=False)

    x_d = nc.dram_tensor("x", [NC, P, TC, F], f32, kind="ExternalInput").ap()
    o_d = nc.dram_tensor("o", [NC, P, TC, F], odt, kind="ExternalOutput").ap()

    fused = _get_fused_op()

    with tile.TileContext(nc) as tc:
        with (
            tc.tile_pool(name="xp", bufs=X_BUFS) as xp,
            tc.tile_pool(name="up", bufs=U_BUFS) as up,
            tc.tile_pool(name="op", bufs=O_BUFS) as op_,
        ):
            def dma_eng(idx, out=False):
                if DMA_Q == "sync":
                    return nc.sync
                if DMA_Q == "scalar":
                    return nc.scalar
                if DMA_Q == "dir":  # x-in on SP ring, o-out on ACT ring
                    return nc.scalar if out else nc.sync
                return nc.sync if idx % 2 == 0 else nc.scalar

            u_prev = None  # [P, F] slice of the previous chunk's history
            for c in range(NC):
                xt = xp.tile([P, TC, F], f32)
                xs = TC // X_SPLIT
                for s in range(X_SPLIT):
                    dma_eng(c).dma_start(
                        out=xt[:, s * xs:(s + 1) * xs, :],
                        in_=x_d[c][:, s * xs:(s + 1) * xs, :],
                    )
                uh = up.tile([P, TC, F], f32)   # membrane history for chunk
                ot = op_.tile([P, TC, F], odt)

                for tl in range(TC):
                    u_new = uh[:, tl, :]
                    if c == 0 and tl == 0:
                        # u_0 = x_0 (zero carry)
                        nc.vector.tensor_copy(u_new, xt[:, 0, :])
                    else:
                        nc.vector._custom_dve(
                            fused,
                            out=u_new,
                            in0=u_prev,
                            in1=xt[:, tl, :],
                            s0=VTH,
                            s1=TAU,
                        )
                    u_prev = u_new
                    if not SPIKE_CHUNK:
                        nc.vector.tensor_scalar(
                            ot[:, tl, :], u_new, VTH, None, mybir.AluOpType.is_gt
                        )
                if SPIKE_CHUNK:
                    # one strict-compare over the whole chunk history
                    nc.vector.tensor_scalar(
                        ot[:], uh[:], VTH, None, mybir.AluOpType.is_gt
                    )
                os_ = TC // O_SPLIT
                for s in range(O_SPLIT):
                    dma_eng(c + 1, out=True).dma_start(
                        out=o_d[c][:, s * os_:(s + 1) * os_, :],
                        in_=ot[:, s * os_:(s + 1) * os_, :],
                    )
    nc.compile()
    return nc


def kernel(x, ksi=None, trace=False):
    """Full-input entry: x [16,64,32,32,50] f32 -> spikes, same shape.
    (ksi is unused by the reference computation.)"""
    global LAST_RESULTS
    x = np.ascontiguousarray(np.asarray(x, dtype=np.float32))
    orig_shape = x.shape
    xf = x.reshape(S_FULL, T)

    nc = _build_program()

    # device layout per core: [chunk, partition, t-in-chunk, free-spatial]
    in_maps = []
    for i in range(N_CORES):
        xc = xf[i * S_CORE:(i + 1) * S_CORE]            # [S_CORE, T]
        xd = xc.reshape(P, F, NC, TC).transpose(2, 0, 3, 1)  # [NC, P, TC, F]
        in_maps.append({"x": np.ascontiguousarray(xd)})

    res = run_bass_kernel_spmd(nc, in_maps, list(range(N_CORES)), trace=trace)
    LAST_RESULTS = res

    out = np.empty((S_FULL, T), dtype=np.float32)
    for i in range(N_CORES):
        r = res.results[i]["o"]                          # [NC, P, TC, F]
        oc = np.asarray(r).transpose(1, 3, 0, 2).reshape(S_CORE, T)
        if oc.dtype != np.float32:
            oc = (oc != 0).astype(np.float32) if O_DT == "u8" else oc.astype(np.float32)
        out[i * S_CORE:(i + 1) * S_CORE] = oc
    return out.reshape(orig_shape)
